# revision 1
# baseline (speedup 1.0000x reference)
"""Trainium2 Bass kernel for nn_GAT_22462678958399.

Dense-GAT attention + MLP head, data-parallel over the 4096-graph batch
across 8 NeuronCores (512 graphs/core), small weights replicated.

Math (per graph, N=116 nodes):
    v12  = W_att @ [a1, a2]                      (host, f64->f32: [116, 2])
    F    = x[g] @ v12                            (PE; f1, f2 per node)
    z    = f1[i] + f2[j] + mask                  (DVE, mask = -60000 where adj==0)
    e    = max(z, 0.2*z)                         (DVE fused scalar_tensor_tensor)
    p    = exp(e)                                (ACT; bf16 out)
    s_i  = sum_j p                               (PE: 0/1 selector matmuls on p^T)
    att  = p * (1/s)                             (DVE; bf16)
    h1   = relu(att_flat @ W1 + b1)              (PE bf16 + ACT)
    h2   = h1 @ W2 + b2                          (PE)
    out  = log_softmax(h2)                       (DVE/ACT tail)

Quantization (validated vs f64 ground truth, output absmax ~2.6e-4 on a
0.73-absmax output): x bf16 (host-pretransposed load), F/z/e fp16,
p/att/W1/W2/h1 bf16, accumulations and reductions f32 (PSUM), r=1/s f32.

Measured ~580-640 us on 8 axon TRN2 cores (64-iteration hardware-loop
delta on device-resident inputs; per-engine busy from the cost model:
DVE ~235 us, PE ~233 us, DMA ~231 us, ACT ~115 us).
"""

import numpy as np
import ml_dtypes

import concourse.bass as bass
import concourse.bacc as bacc
import concourse.mybir as mybir
import concourse.tile as tile
from concourse.bass_utils import run_bass_kernel_spmd

BF16 = ml_dtypes.bfloat16

N = 116            # nodes per graph
NN = N * N         # 13456
B = 4096
NCORES = 8
G = 512            # graphs per core
NKC = 106          # k chunks of 128 (13568 = 106*128)
KPAD = NKC * 128   # 13568
GH = 128           # graphs per half-tile
NHALF = G // GH    # 4
NST = 16           # x sub-tiles per core (32 graphs each)
STG = 32           # graphs per sub-tile
STR = STG * N      # 3712 rows per sub-tile
OC = 8             # output chunks of 128 (1024 = 8*128)
NSTRIP = 4         # z strips per half
SW = NN // NSTRIP  # 6728 = 58*116
SI = N // NSTRIP   # 58 i-rows per strip
MASKVAL = -60000.0

f32 = mybir.dt.float32
bf16 = mybir.dt.bfloat16
fp16 = mybir.dt.float16

AL = mybir.AluOpType
AF = mybir.ActivationFunctionType
AX = mybir.AxisListType


def build_nc(reps=1):
    nc = bacc.Bacc("TRN2", target_bir_lowering=False, debug=False)

    X = nc.dram_tensor("x", [NST, N, STR], bf16, kind="ExternalInput")
    MADJ = nc.dram_tensor("madj", [NHALF, GH, NN], fp16, kind="ExternalInput")
    W1 = nc.dram_tensor("w1", [OC, 128, NKC * 128], bf16, kind="ExternalInput")
    W2 = nc.dram_tensor("w2", [128, 16], bf16, kind="ExternalInput")
    B1 = nc.dram_tensor("b1r", [128, OC], f32, kind="ExternalInput")
    B2 = nc.dram_tensor("b2c", [1, 2], f32, kind="ExternalInput")
    VV = nc.dram_tensor("vvb", [N, 2], bf16, kind="ExternalInput")
    I128 = nc.dram_tensor("i128b", [128, 128], bf16, kind="ExternalInput")
    I116 = nc.dram_tensor("i116h", [N, N], fp16, kind="ExternalInput")
    OUT = nc.dram_tensor("out", [2, G], f32, kind="ExternalOutput")

    HS = NKC // 2  # 53 chunks per W1 half-slab

    from contextlib import ExitStack
    with tile.TileContext(nc) as tc:
        with ExitStack() as es:
            pool = lambda name, bufs, space="SBUF": es.enter_context(
                tc.tile_pool(name=name, bufs=bufs, space=space))
            cpool = pool("const", 1)
            apool = pool("attT", 1)
            fpool = pool("fsmall", 2)
            xpool = pool("xT", 1)
            fhpool = pool("fh", 1)
            zpool = pool("z", 2)
            mpool = pool("msb", 2)
            ppool = pool("p", 1)
            srpool = pool("sr", 1)
            wpool = pool("w1sb", 2)
            hpool = pool("h1", 1)
            tpool = pool("tail", 1)
            psF = pool("psF", 1, "PSUM")
            psFg = pool("psFg", 1, "PSUM")
            psT = pool("psT", 2, "PSUM")
            psH = pool("psH", 2, "PSUM")
            psH2 = pool("psH2", 1, "PSUM")

            i128 = cpool.tile_from(I128[:])
            i116 = cpool.tile_from(I116[:])
            vvb = cpool.tile_from(VV[:])
            b1s = cpool.tile_from(B1[:])
            b2s = cpool.tile_from(B2[:])
            w2s = cpool.tile([128, 16], bf16, tag="w2s", name="w2s")
            nc.gpsimd.dma_start(w2s[:], W2[:])

            # attention^T per batch-tile of 256 graphs: [128 k, c-major, 256 g]
            attTb = [apool.tile([128, NKC * 256], bf16, tag=f"attT{b}",
                                name=f"attT{b}") for b in range(2)]
            ca = tpool.tile([1, G], f32, tag="ca", name="ca")
            cb = tpool.tile([1, G], f32, tag="cb", name="cb")
            t6 = tpool.tile([1, G], f32, tag="t6", name="t6")

            def emit_body():
              for bt in range(2):
                  attT = attTb[bt]
                  av = attT.rearrange("q (c g) -> q c g", g=256)
                  for hh in range(2):
                      h = 2 * bt + hh
                      # ---- F = x @ v12 for this half (128 graphs) ----
                      fp = psF.tile([N, 256], f32, tag="fp")
                      for sub in range(4):
                          st = 4 * h + sub
                          xT = xpool.tile([N, STR], bf16, tag="xT")
                          nc.gpsimd.dma_start(xT[:], X[st])
                          for g in range(STG):
                              gg = sub * STG + g
                              nc.tensor.matmul(
                                  fp[:, 2 * gg:2 * gg + 2],
                                  xT[:, N * g:N * g + N],
                                  vvb[:],
                                  start=True, stop=True,
                              )
                      fh = fhpool.tile([N, 256], fp16, tag="fh")
                      nc.scalar.copy(fh[:], fp[:])
                      fgp = psFg.tile([128, 2 * N], fp16, tag="fgp")
                      nc.tensor.transpose(fgp[:, 0:N], fh[:, 0::2], i116[:])
                      nc.tensor.transpose(fgp[:, N:2 * N], fh[:, 1::2], i116[:])
                      f1t = fpool.tile([128, N], fp16, tag="f1t", name="f1t")
                      f2t = fpool.tile([128, N], fp16, tag="f2t", name="f2t")
                      nc.vector.tensor_copy(f1t[:], fgp[:, 0:N])
                      nc.vector.tensor_copy(f2t[:], fgp[:, N:2 * N])

                      # ---- z, lrelu, exp, per-strip softmax+transpose ----
                      p = ppool.tile([128, KPAD], bf16, tag="p")
                      nc.vector.memset(p[:, NN:KPAD], 0.0)
                      sb = srpool.tile([128, N], f32, tag="sb")
                      rb = srpool.tile([128, N], f32, tag="rb")
                      gofs = 128 * hh
                      done = 0
                      for s in range(NSTRIP):
                          z = zpool.tile([128, SW], fp16, tag="z")
                          msb = mpool.tile([128, SW], fp16, tag="msb")
                          nc.gpsimd.dma_start(msb[:], MADJ[h][:, s * SW:(s + 1) * SW])
                          f1v = (f1t[:, s * SI:(s + 1) * SI]
                                 .rearrange("q (i u) -> q i u", u=1)
                                 .broadcast_to([128, SI, N]))
                          f2v = (f2t[:]
                                 .rearrange("q (u j) -> q u j", u=1)
                                 .broadcast_to([128, SI, N]))
                          zv = z.rearrange("q (i j) -> q i j", j=N)
                          nc.vector.tensor_tensor(zv, f1v, f2v, op=AL.add)
                          nc.vector.tensor_tensor(z[:], z[:], msb[:], op=AL.add)
                          nc.vector.scalar_tensor_tensor(
                              z[:], z[:], 0.2, z[:], op0=AL.mult, op1=AL.max)
                          ps = p[:, s * SW:(s + 1) * SW]
                          nc.scalar.activation(ps, z[:], AF.Exp)
                          # segmented sums + recip + normalize for this strip
                          pvs = ps.rearrange("q (i j) -> q i j", j=N)
                          sbs = sb[:, s * SI:(s + 1) * SI]
                          rbs = rb[:, s * SI:(s + 1) * SI]
                          nc.vector.tensor_reduce(sbs, pvs, axis=AX.X, op=AL.add)
                          nc.vector.reciprocal(rbs, sbs)
                          rvs = (rbs.rearrange("q (i u) -> q i u", u=1)
                                 .broadcast_to([128, SI, N]))
                          nc.vector.tensor_tensor(pvs, pvs, rvs, op=AL.mult)
                          # transpose + evac all chunks fully covered so far
                          lim = ((s + 1) * SW) // 128 if s < NSTRIP - 1 else NKC
                          while done < lim:
                              hi = min(done + 8, lim)
                              tp = psT.tile([128, 1024], bf16, tag="tp")
                              for c in range(done, hi):
                                  nc.tensor.transpose(
                                      tp[:, 128 * (c - done):128 * (c - done) + 128],
                                      p[:, 128 * c:128 * c + 128],
                                      i128[:])
                              dstv = av[:, done:hi, gofs:gofs + 128]
                              srcv = tp.rearrange(
                                  "q (c g) -> q c g", g=128)[:, 0:hi - done, :]
                              nc.scalar.copy(dstv, srcv)
                              done = hi

                  # ---- MLP pass for this batch-tile (256 graphs) ----
                  h2a = psH2.tile([1, 256], f32, tag="h2a", name="h2a")
                  h2b = psH2.tile([1, 256], f32, tag="h2b", name="h2b")
                  for oc in range(OC):
                      hp = psH.tile([128, 256], f32, tag="hp")
                      for hs in range(2):
                          w1s = wpool.tile([128, HS * 128], bf16, tag="w1s")
                          nc.gpsimd.dma_start(
                              w1s[:], W1[oc][:, hs * HS * 128:(hs + 1) * HS * 128])
                          for c in range(HS):
                              cc = hs * HS + c
                              nc.tensor.matmul(
                                  hp[:],
                                  w1s[:, 128 * c:128 * c + 128],
                                  attT[:, 256 * cc:256 * cc + 256],
                                  start=(cc == 0), stop=(cc == NKC - 1),
                              )
                      h1 = hpool.tile([128, 256], bf16, tag="h1", bufs=2, name="h1")
                      nc.scalar.activation(
                          h1[:], hp[:], AF.Relu,
                          bias=b1s[:, oc:oc + 1], scale=1.0)
                      nc.tensor.matmul(
                          h2a[:], w2s[:, 2 * oc:2 * oc + 1], h1[:],
                          start=(oc == 0), stop=(oc == OC - 1),
                      )
                      nc.tensor.matmul(
                          h2b[:], w2s[:, 2 * oc + 1:2 * oc + 2], h1[:],
                          start=(oc == 0), stop=(oc == OC - 1),
                      )
                  nc.scalar.activation(ca[:, 256 * bt:256 * bt + 256], h2a[:],
                                       AF.Identity, bias=b2s[0:1, 0:1], scale=1.0)
                  nc.scalar.activation(cb[:, 256 * bt:256 * bt + 256], h2b[:],
                                       AF.Identity, bias=b2s[0:1, 1:2], scale=1.0)

              # ---- log_softmax over the 2 classes ----
              nc.vector.tensor_tensor(t6[:], ca[:], cb[:], op=AL.max)
              nc.vector.tensor_tensor(ca[:], ca[:], t6[:], op=AL.subtract)
              nc.vector.tensor_tensor(cb[:], cb[:], t6[:], op=AL.subtract)
              ea = psH.tile([1, G], f32, tag="hp", name="ea")
              nc.scalar.activation(ea[:], ca[:], AF.Exp)
              nc.scalar.activation(t6[:], cb[:], AF.Exp)
              nc.vector.tensor_tensor(t6[:], t6[:], ea[:], op=AL.add)
              nc.scalar.activation(t6[:], t6[:], AF.Ln)
              nc.vector.tensor_tensor(ca[:], ca[:], t6[:], op=AL.subtract)
              nc.vector.tensor_tensor(cb[:], cb[:], t6[:], op=AL.subtract)
              nc.gpsimd.dma_start(OUT[0:1, :], ca[:])
              nc.gpsimd.dma_start(OUT[1:2, :], cb[:])

            if reps == 1:
                emit_body()
            else:
                with tc.For_i(0, reps, 1) as _i:
                    emit_body()

    return nc


def _prep_inputs(x, adj, W_att, a1, a2, W1, b1, W2, b2):
    x = np.asarray(x, np.float32)
    adj = np.asarray(adj, np.float32)
    xbf = x.astype(BF16)
    xr = np.ascontiguousarray(
        xbf.reshape(NCORES, NST, STG, N, N).transpose(0, 1, 4, 2, 3)
        .reshape(NCORES, NST, N, STR))
    madj = np.where(adj > 0, np.float16(0.0), np.float16(MASKVAL))
    madjr = np.ascontiguousarray(
        madj.reshape(NCORES, NHALF, GH, NN)).astype(np.float16)
    v12 = (np.asarray(W_att, np.float64)
           @ np.stack([np.asarray(a1, np.float64), np.asarray(a2, np.float64)], 1))
    vvb = np.ascontiguousarray(v12.astype(np.float32).astype(BF16))
    W1p = np.zeros((KPAD, 1024), np.float32)
    W1p[:NN] = np.asarray(W1, np.float32)
    w1r = np.ascontiguousarray(
        W1p.reshape(NKC, 128, OC, 128).transpose(2, 1, 0, 3)
        .reshape(OC, 128, NKC * 128)).astype(BF16)
    w2r = np.ascontiguousarray(
        np.asarray(W2, np.float32).reshape(OC, 128, 2)
        .transpose(1, 0, 2).reshape(128, 16)).astype(BF16)
    b1r = np.ascontiguousarray(np.asarray(b1, np.float32).reshape(OC, 128).T)
    b2c = np.ascontiguousarray(np.asarray(b2, np.float32).reshape(1, 2))
    consts = dict(
        w1=w1r, w2=w2r, b1r=b1r, b2c=b2c, vvb=vvb,
        i128b=np.eye(128, dtype=BF16),
        i116h=np.eye(N, dtype=np.float16),
    )
    return [dict(consts, x=xr[c], madj=madjr[c]) for c in range(NCORES)]


TRACE = False
LAST_RESULTS = None


def kernel(x, adj, W_att, a1, a2, W1, b1, W2, b2):
    global LAST_RESULTS
    in_maps = _prep_inputs(x, adj, W_att, a1, a2, W1, b1, W2, b2)
    nc = build_nc()
    nc.compile()
    bres = run_bass_kernel_spmd(nc, in_maps, list(range(NCORES)), trace=TRACE)
    LAST_RESULTS = bres
    res = bres.results
    out = np.empty((B, 2), np.float32)
    for c in range(NCORES):
        out[c * G:(c + 1) * G] = np.asarray(res[c]["out"]).T
    return out



# revision 17
# speedup vs baseline: 1.4600x; 1.4600x over previous
"""Trainium2 Bass kernel for nn_GAT_22462678958399.

Dense-GAT attention + MLP head, data-parallel over the 4096-graph batch
across 8 NeuronCores (512 graphs/core), small weights replicated.

Math (per graph, N=116 nodes):
    v12  = W_att @ [a1, a2]                      (host, f64->f32: [116, 2])
    F    = x[g] @ v12                            (PE; f1, f2 per node; x fp8)
    z    = f1[i] + f2[j]                         (Pool/GPSIMD broadcast add)
    u    = z + mask                              (DVE, mask = -60000 where adj==0)
    p    = max(exp(u), exp(0.2u)) = exp(lrelu(u))  (ACT x2 + DVE max)
    s_i  = sum_j p                               (DVE tree-sum, fp16)
    att  = p * (16/s)                            (DVE/Pool; fp8 out, x16 scaled)
    h1   = relu((att16 @ W1x64)/1024 + b1)       (PE fp8 DoubleRow + ACT)
    h2   = h1 @ W2 + b2                          (PE bf16)
    out  = log_softmax(h2)                       (DVE/ACT tail)

Engine plan (cost-model busy per core): DVE ~150us (mask/max/tree/norm/evac),
Pool ~135us (z + some norm), ACT ~135us (exps, evac share, relu), PE ~120us
(fp8 DoubleRow MLP + transposes), DMA ~135us (x fp8, madj fp16, W1 fp8 x2).
All DMA issue goes through the idle sync (SP) queue.
"""

import numpy as np
import ml_dtypes

import concourse.bass as bass
import concourse.bacc as bacc
import concourse.mybir as mybir
import concourse.tile as tile
from concourse.bass_utils import run_bass_kernel_spmd

BF16 = ml_dtypes.bfloat16
FP8 = ml_dtypes.float8_e4m3

N = 116            # nodes per graph
NN = N * N         # 13456
B = 4096
NCORES = 8
G = 512            # graphs per core
NKC = 106          # k chunks of 128 (13568 = 106*128)
KPAD = NKC * 128   # 13568
GH = 128           # graphs per half-tile
NHALF = G // GH    # 4
NST = 16           # x sub-tiles per core (32 graphs each)
STG = 32           # graphs per sub-tile
STR = STG * N      # 3712 rows per sub-tile
OC = 8             # output chunks of 128 (1024 = 8*128)
NSTRIP = 4         # z strips per half
SW = NN // NSTRIP  # 3364 = 29*116
SI = N // NSTRIP   # 29 i-rows per strip
MASKVAL = -60000.0
W1SCALE = 64.0     # host-side W1 scale into fp8 normal range
ATTSCALE = 16.0    # att scale into fp8 normal range
H1SCALE = 1.0 / (W1SCALE * ATTSCALE)

f32 = mybir.dt.float32
bf16 = mybir.dt.bfloat16
fp16 = mybir.dt.float16
fp8 = mybir.dt.float8e4

AL = mybir.AluOpType
AF = mybir.ActivationFunctionType
AX = mybir.AxisListType
PM = mybir.MatmulPerfMode

# ---- engine assignment knobs ----
# norm strips handled by Pool (others on DVE), per (half, strip)
NORM_POOL = {(h, 3) for h in range(4)} | {(0, 2), (2, 2)}
# evac batches: DVE for 2 of every 7 batches, else ACT
EVAC_DVE = {5, 6}

# W1 half-slab chunk splits (must pair chunks for fp8 DoubleRow)
SLABS = [(0, 27), (54, 26)]  # (first chunk, npairs); 27*2=54, 26*2=52 chunks


def build_nc(reps=1):
    nc = bacc.Bacc("TRN2", target_bir_lowering=False, debug=False)

    X = nc.dram_tensor("x", [NST, N, STR], fp8, kind="ExternalInput")
    MADJ = nc.dram_tensor("madj", [NHALF, GH, NN], fp16, kind="ExternalInput")
    W1 = nc.dram_tensor("w1", [OC, 128, NKC * 128], fp8, kind="ExternalInput")
    W2 = nc.dram_tensor("w2", [128, 16], bf16, kind="ExternalInput")
    B1 = nc.dram_tensor("b1r", [128, OC], f32, kind="ExternalInput")
    B2 = nc.dram_tensor("b2c", [1, 2], f32, kind="ExternalInput")
    VV = nc.dram_tensor("vvb", [N, 2], bf16, kind="ExternalInput")
    I128 = nc.dram_tensor("i128b", [128, 128], fp8, kind="ExternalInput")
    I116 = nc.dram_tensor("i116h", [N, N], fp16, kind="ExternalInput")
    OUT = nc.dram_tensor("out", [2, G], f32, kind="ExternalOutput")

    from contextlib import ExitStack
    with tile.TileContext(nc) as tc:
        with ExitStack() as es:
            pool = lambda name, bufs, space="SBUF": es.enter_context(
                tc.tile_pool(name=name, bufs=bufs, space=space))
            cpool = pool("const", 1)
            apool = pool("attT", 1)
            fpool = pool("fsmall", 2)
            xpool = pool("xT", 2)
            fhpool = pool("fh", 2)
            zpool = pool("z", 2)
            mpool = pool("msb", 2)
            ppool = pool("p", 2)
            pnpool = pool("pn", 1)
            t1pool = pool("t1", 1)
            t2pool = pool("t2", 1)
            srpool = pool("sr", 2)
            wpool = pool("w1sb", 2)
            hpool = pool("h1", 2)
            tpool = pool("tail", 1)
            psF = pool("psF", 1, "PSUM")
            psFg = pool("psFg", 1, "PSUM")
            psT = pool("psT", 2, "PSUM")
            psH = pool("psH", 2, "PSUM")
            psH2 = pool("psH2", 1, "PSUM")

            i128 = cpool.tile_from(I128[:])
            i116 = cpool.tile_from(I116[:])
            vvb = cpool.tile_from(VV[:])
            b1s = cpool.tile_from(B1[:])
            b2s = cpool.tile_from(B2[:])
            w2s = cpool.tile([128, 16], bf16, tag="w2s", name="w2s")
            nc.sync.dma_start(w2s[:], W2[:])

            # attention^T per batch-tile of 256 graphs: [128 k, c-major, 256 g]
            attTb = [apool.tile([128, NKC * 256], fp8, tag=f"attT{b}",
                                name=f"attT{b}") for b in range(2)]
            ca = tpool.tile([1, G], f32, tag="ca", name="ca")
            cb = tpool.tile([1, G], f32, tag="cb", name="cb")
            t6 = tpool.tile([1, G], f32, tag="t6", name="t6")

            def emit_body():
              for bt in range(2):
                  attT = attTb[bt]
                  av = attT.rearrange("q (c g) -> q c g", g=256)
                  for hh in range(2):
                      h = 2 * bt + hh
                      # ---- F = x @ v12 for this half (128 graphs) ----
                      fp = psF.tile([N, 256], f32, tag="fp")
                      for sub in range(4):
                          st = 4 * h + sub
                          xT = xpool.tile([N, STR], fp8, tag="xT")
                          nc.sync.dma_start(xT[:], X[st])
                          for g in range(STG):
                              gg = sub * STG + g
                              nc.tensor.matmul(
                                  fp[:, 2 * gg:2 * gg + 2],
                                  xT[:, N * g:N * g + N],
                                  vvb[:],
                                  start=True, stop=True,
                              )
                      fh = fhpool.tile([N, 256], fp16, tag="fh")
                      nc.scalar.copy(fh[:], fp[:])
                      fgp = psFg.tile([128, 2 * N], fp16, tag="fgp")
                      nc.tensor.transpose(fgp[:, 0:N], fh[:, 0::2], i116[:])
                      nc.tensor.transpose(fgp[:, N:2 * N], fh[:, 1::2], i116[:])
                      f1t = fpool.tile([128, N], fp16, tag="f1t", name="f1t")
                      f2t = fpool.tile([128, N], fp16, tag="f2t", name="f2t")
                      nc.vector.tensor_copy(f1t[:], fgp[:, 0:N])
                      nc.vector.tensor_copy(f2t[:], fgp[:, N:2 * N])

                      # ---- z, mask, exp-domain lrelu ----
                      p = ppool.tile([128, NN], fp16, tag="p")
                      for s in range(NSTRIP):
                          msb = mpool.tile([128, SW], fp16, tag="msb")
                          nc.sync.dma_start(msb[:], MADJ[h][:, s * SW:(s + 1) * SW])
                          z = zpool.tile([128, SW], fp16, tag="z")
                          f1v = (f1t[:, s * SI:(s + 1) * SI]
                                 .rearrange("q (i u) -> q i u", u=1)
                                 .broadcast_to([128, SI, N]))
                          f2v = (f2t[:]
                                 .rearrange("q (u j) -> q u j", u=1)
                                 .broadcast_to([128, SI, N]))
                          zv = z.rearrange("q (i j) -> q i j", j=N)
                          nc.gpsimd.tensor_tensor(zv, f1v, f2v, op=AL.add)
                          nc.vector.tensor_tensor(z[:], z[:], msb[:], op=AL.add)
                          ps = p[:, s * SW:(s + 1) * SW]
                          nc.scalar.activation(ps, z[:], AF.Exp)
                          nc.scalar.activation(z[:], z[:], AF.Exp, scale=0.2)
                          nc.vector.tensor_tensor(ps, ps, z[:], op=AL.max)

                      # ---- per-strip: fp16 tree-sum, 16/s recip, fp8 norm ----
                      pv = p.rearrange("q (i j) -> q i j", j=N)
                      pn = pnpool.tile([128, KPAD], fp8, tag="pn")
                      nc.gpsimd.memset(pn[:, NN:KPAD], 0.0)
                      sb = srpool.tile([128, N], f32, tag="sb", name="sb")
                      rb = srpool.tile([128, N], f32, tag="rb", name="rb")
                      for s in range(NSTRIP):
                          i0, i1 = s * SI, (s + 1) * SI
                          pv_s = pv[:, i0:i1, :]
                          t1 = t1pool.tile([128, SI * 58], fp16, tag="t1")
                          t1v = t1.rearrange("q (i j) -> q i j", j=58)
                          nc.vector.tensor_tensor(
                              t1v, pv_s[:, :, 0:58], pv_s[:, :, 58:116],
                              op=AL.add)
                          t2 = t2pool.tile([128, SI * 29], fp16, tag="t2")
                          t2v = t2.rearrange("q (i j) -> q i j", j=29)
                          nc.vector.tensor_tensor(
                              t2v, t1v[:, :, 0:29], t1v[:, :, 29:58], op=AL.add)
                          nc.vector.tensor_tensor(
                              t1v[:, :, 0:14], t2v[:, :, 0:14], t2v[:, :, 15:29],
                              op=AL.add)
                          nc.vector.tensor_copy(
                              t1v[:, :, 14:15], t2v[:, :, 14:15])
                          sbs = sb[:, i0:i1]
                          rbs = rb[:, i0:i1]
                          nc.vector.tensor_reduce(
                              sbs, t1v[:, :, 0:15], axis=AX.X, op=AL.add)
                          nc.vector.tensor_scalar_mul(sbs, sbs, 1.0 / ATTSCALE)
                          nc.vector.reciprocal(rbs, sbs)
                          pnv_s = (pn[:, s * SW:(s + 1) * SW]
                                   .rearrange("q (i j) -> q i j", j=N))
                          rv = (rbs.rearrange("q (i u) -> q i u", u=1)
                                .broadcast_to([128, SI, N]))
                          eng = nc.gpsimd if (h, s) in NORM_POOL else nc.vector
                          eng.tensor_tensor(pnv_s, pv_s, rv, op=AL.mult)

                      # ---- transpose chunks + evac into attT ----
                      # fp8 PE transpose writes with element step 2 at 4B-
                      # aligned starts (hw rule): each chunk gets its own 256B
                      # PSUM region, values at even bytes; evac copies do the
                      # stride removal into dense fp8 attT.
                      gofs = 128 * hh
                      done = 0
                      bidx = 0
                      while done < NKC:
                          hi = min(done + 8, NKC)
                          tp = psT.tile([128, 2048], fp8, tag="tp")
                          tv = tp.rearrange("q (c b) -> q c b", b=256)
                          for c in range(done, hi):
                              nc.tensor.transpose(
                                  tv[:, c - done, 0::2],
                                  pn[:, 128 * c:128 * c + 128],
                                  i128[:])
                          dstv = av[:, done:hi, gofs:gofs + 128]
                          srcv = tv[:, 0:hi - done, 0::2]
                          if bidx % 7 in EVAC_DVE:
                              nc.vector.tensor_copy(dstv, srcv)
                          else:
                              nc.scalar.copy(dstv, srcv)
                          done = hi
                          bidx += 1

                  # ---- MLP pass for this batch-tile (256 graphs) ----
                  h2a = psH2.tile([1, 256], f32, tag="h2a", name="h2a")
                  h2b = psH2.tile([1, 256], f32, tag="h2b", name="h2b")
                  for oc in range(OC):
                      hp = psH.tile([128, 256], f32, tag="hp")
                      for (c0, npair) in SLABS:
                          w1s = wpool.tile([128, npair * 2 * 128], fp8, tag="w1s")
                          nc.scalar.dma_start(
                              w1s[:],
                              W1[oc][:, c0 * 128:(c0 + 2 * npair) * 128])
                          w1v = w1s.rearrange("q (c m) -> q c m", m=128)
                          for pr in range(npair):
                              cc = c0 + 2 * pr
                              nc.tensor.matmul(
                                  hp[:],
                                  w1v[:, 2 * pr:2 * pr + 2, :],
                                  av[:, cc:cc + 2, :],
                                  start=(cc == 0), stop=(cc + 2 == NKC),
                                  perf_mode=PM.DoubleRow,
                              )
                      h1 = hpool.tile([128, 256], bf16, tag="h1", name="h1")
                      nc.scalar.activation(
                          h1[:], hp[:], AF.Relu,
                          bias=b1s[:, oc:oc + 1], scale=H1SCALE)
                      nc.tensor.matmul(
                          h2a[:], w2s[:, 2 * oc:2 * oc + 1], h1[:],
                          start=(oc == 0), stop=(oc == OC - 1),
                      )
                      nc.tensor.matmul(
                          h2b[:], w2s[:, 2 * oc + 1:2 * oc + 2], h1[:],
                          start=(oc == 0), stop=(oc == OC - 1),
                      )
                  nc.scalar.activation(ca[:, 256 * bt:256 * bt + 256], h2a[:],
                                       AF.Identity, bias=b2s[0:1, 0:1], scale=1.0)
                  nc.scalar.activation(cb[:, 256 * bt:256 * bt + 256], h2b[:],
                                       AF.Identity, bias=b2s[0:1, 1:2], scale=1.0)

              # ---- log_softmax over the 2 classes ----
              nc.vector.tensor_tensor(t6[:], ca[:], cb[:], op=AL.max)
              nc.vector.tensor_tensor(ca[:], ca[:], t6[:], op=AL.subtract)
              nc.vector.tensor_tensor(cb[:], cb[:], t6[:], op=AL.subtract)
              ea = psH.tile([1, G], f32, tag="hp", name="ea")
              nc.scalar.activation(ea[:], ca[:], AF.Exp)
              nc.scalar.activation(t6[:], cb[:], AF.Exp)
              nc.vector.tensor_tensor(t6[:], t6[:], ea[:], op=AL.add)
              nc.scalar.activation(t6[:], t6[:], AF.Ln)
              nc.vector.tensor_tensor(ca[:], ca[:], t6[:], op=AL.subtract)
              nc.vector.tensor_tensor(cb[:], cb[:], t6[:], op=AL.subtract)
              nc.sync.dma_start(OUT[0:1, :], ca[:])
              nc.sync.dma_start(OUT[1:2, :], cb[:])

            if reps == 1:
                emit_body()
            else:
                with tc.For_i(0, reps, 1) as _i:
                    emit_body()

    return nc


def _prep_inputs(x, adj, W_att, a1, a2, W1, b1, W2, b2):
    x = np.asarray(x, np.float32)
    adj = np.asarray(adj, np.float32)
    xf8 = x.astype(FP8)
    xr = np.ascontiguousarray(
        xf8.reshape(NCORES, NST, STG, N, N).transpose(0, 1, 4, 2, 3)
        .reshape(NCORES, NST, N, STR))
    madj = np.where(adj > 0, np.float16(0.0), np.float16(MASKVAL))
    madjr = np.ascontiguousarray(
        madj.reshape(NCORES, NHALF, GH, NN)).astype(np.float16)
    v12 = (np.asarray(W_att, np.float64)
           @ np.stack([np.asarray(a1, np.float64), np.asarray(a2, np.float64)], 1))
    vvb = np.ascontiguousarray(v12.astype(np.float32).astype(BF16))
    W1p = np.zeros((KPAD, 1024), np.float32)
    W1p[:NN] = np.asarray(W1, np.float32) * W1SCALE
    w1r = np.ascontiguousarray(
        W1p.reshape(NKC, 128, OC, 128).transpose(2, 1, 0, 3)
        .reshape(OC, 128, NKC * 128)).astype(FP8)
    w2r = np.ascontiguousarray(
        np.asarray(W2, np.float32).reshape(OC, 128, 2)
        .transpose(1, 0, 2).reshape(128, 16)).astype(BF16)
    b1r = np.ascontiguousarray(np.asarray(b1, np.float32).reshape(OC, 128).T)
    b2c = np.ascontiguousarray(np.asarray(b2, np.float32).reshape(1, 2))
    consts = dict(
        w1=w1r, w2=w2r, b1r=b1r, b2c=b2c, vvb=vvb,
        i128b=np.eye(128, dtype=FP8),
        i116h=np.eye(N, dtype=np.float16),
    )
    return [dict(consts, x=xr[c], madj=madjr[c]) for c in range(NCORES)]


TRACE = False
LAST_RESULTS = None


def kernel(x, adj, W_att, a1, a2, W1, b1, W2, b2):
    global LAST_RESULTS
    in_maps = _prep_inputs(x, adj, W_att, a1, a2, W1, b1, W2, b2)
    nc = build_nc()
    nc.compile()
    bres = run_bass_kernel_spmd(nc, in_maps, list(range(NCORES)), trace=TRACE)
    LAST_RESULTS = bres
    res = bres.results
    out = np.empty((B, 2), np.float32)
    for c in range(NCORES):
        out[c * G:(c + 1) * G] = np.asarray(res[c]["out"]).T
    return out


# revision 23
# speedup vs baseline: 1.9516x; 1.3367x over previous
"""Trainium2 Bass kernel for nn_GAT_22462678958399.

Dense-GAT attention + MLP head, data-parallel over the 4096-graph batch
across 8 NeuronCores (512 graphs/core), small weights replicated.

Math (per graph, N=116 nodes):
    v12  = W_att @ [a1, a2]                      (host, f64->f32: [116, 2])
    F    = x[g] @ v12                            (PE; f1, f2 per node; x fp8)
    z    = f1[i] + f2[j]                         (Pool/GPSIMD broadcast add)
    u    = z + mask                              (DVE, mask = -60000 where adj==0)
    p    = max(exp(u), exp(0.2u)) = exp(lrelu(u))  (ACT x2 + DVE max)
    s_i  = sum_j p                               (DVE tree-sum, fp16)
    att  = p * (16/s)                            (DVE/Pool; fp8 out, x16 scaled)
    h1   = relu((att16 @ W1x64)/1024 + b1)       (PE fp8 DoubleRow + ACT)
    h2   = h1 @ W2 + b2                          (PE bf16)
    out  = log_softmax(h2)                       (DVE/ACT tail)

Engine plan (cost-model busy per core): DVE ~150us (mask/max/tree/norm/evac),
Pool ~135us (z + some norm), ACT ~135us (exps, evac share, relu), PE ~120us
(fp8 DoubleRow MLP + transposes), DMA ~135us (x fp8, madj fp16, W1 fp8 x2).
All DMA issue goes through the idle sync (SP) queue.
"""

import numpy as np
import ml_dtypes

import concourse.bass as bass
import concourse.bacc as bacc
import concourse.mybir as mybir
import concourse.tile as tile
from concourse.bass_utils import run_bass_kernel_spmd

BF16 = ml_dtypes.bfloat16
FP8 = ml_dtypes.float8_e4m3

N = 116            # nodes per graph
NN = N * N         # 13456
B = 4096
NCORES = 8
G = 512            # graphs per core
NKC = 106          # k chunks of 128 (13568 = 106*128)
KPAD = NKC * 128   # 13568
GH = 128           # graphs per half-tile
NHALF = G // GH    # 4
NST = 16           # x sub-tiles per core (32 graphs each)
STG = 32           # graphs per sub-tile
STR = STG * N      # 3712 rows per sub-tile
OC = 8             # output chunks of 128 (1024 = 8*128)
NSTRIP = 4         # z strips per half
SW = NN // NSTRIP  # 3364 = 29*116
SI = N // NSTRIP   # 29 i-rows per strip
MASKVAL = -60000.0
W1SCALE = 64.0     # host-side W1 scale into fp8 normal range
ATTSCALE = 16.0    # att scale into fp8 normal range
H1SCALE = 1.0 / (W1SCALE * ATTSCALE)

f32 = mybir.dt.float32
bf16 = mybir.dt.bfloat16
fp16 = mybir.dt.float16
fp8 = mybir.dt.float8e4

AL = mybir.AluOpType
AF = mybir.ActivationFunctionType
AX = mybir.AxisListType
PM = mybir.MatmulPerfMode

# ---- engine assignment knobs ----
# norm strips handled by Pool (others on DVE), per (half, strip)
NORM_POOL = {(h, 3) for h in range(4)} | {(0, 2), (2, 2)}
# evac batches: DVE for 2 of every 7 batches, else ACT
EVAC_DVE = {5, 6}

# W1 half-slab chunk splits (must pair chunks for fp8 DoubleRow)
SLABS = [(0, 27), (54, 26)]  # (first chunk, npairs); 27*2=54, 26*2=52 chunks


def build_nc(reps=1):
    nc = bacc.Bacc("TRN2", target_bir_lowering=False, debug=False)

    X = nc.dram_tensor("x", [NST, N, STR], fp8, kind="ExternalInput")
    MADJ = nc.dram_tensor("madj", [NHALF, GH, NN], fp16, kind="ExternalInput")
    W1 = nc.dram_tensor("w1", [OC, 128, NKC * 128], fp8, kind="ExternalInput")
    W2 = nc.dram_tensor("w2", [128, 16], bf16, kind="ExternalInput")
    B1 = nc.dram_tensor("b1r", [128, OC], f32, kind="ExternalInput")
    B2 = nc.dram_tensor("b2c", [1, 2], f32, kind="ExternalInput")
    VV = nc.dram_tensor("vvb", [N, 2], bf16, kind="ExternalInput")
    I128 = nc.dram_tensor("i128b", [128, 128], fp8, kind="ExternalInput")
    I116 = nc.dram_tensor("i116h", [N, N], fp16, kind="ExternalInput")
    OUT = nc.dram_tensor("out", [2, G], f32, kind="ExternalOutput")

    from contextlib import ExitStack
    with tile.TileContext(nc) as tc:
        with ExitStack() as es:
            pool = lambda name, bufs, space="SBUF": es.enter_context(
                tc.tile_pool(name=name, bufs=bufs, space=space))
            cpool = pool("const", 1)
            apool = pool("attT", 1)
            fpool = pool("fsmall", 2)
            xpool = pool("xT", 2)
            fhpool = pool("fh", 2)
            zpool = pool("z", 2)
            mpool = pool("msb", 2)
            ppool = pool("p", 2)
            pnpool = pool("pn", 1)
            t1pool = pool("t1", 1)
            t2pool = pool("t2", 1)
            srpool = pool("sr", 2)
            wpool = pool("w1sb", 3)
            hpool = pool("h1", 2)
            tpool = pool("tail", 1)
            psF = pool("psF", 1, "PSUM")
            psFg = pool("psFg", 1, "PSUM")
            psT = pool("psT", 2, "PSUM")
            psH = pool("psH", 2, "PSUM")
            psH2 = pool("psH2", 1, "PSUM")

            i128 = cpool.tile_from(I128[:])
            i116 = cpool.tile_from(I116[:])
            vvb = cpool.tile_from(VV[:])
            b1s = cpool.tile_from(B1[:])
            b2s = cpool.tile_from(B2[:])
            w2s = cpool.tile([128, 16], bf16, tag="w2s", name="w2s")
            nc.sync.dma_start(w2s[:], W2[:])

            BTW = [256, 256]
            attTb = [apool.tile([128, NKC * BTW[b]], fp8, tag=f"attT{b}",
                                name=f"attT{b}") for b in range(2)]
            ca = tpool.tile([1, G], f32, tag="ca", name="ca")
            cb = tpool.tile([1, G], f32, tag="cb", name="cb")
            t6 = tpool.tile([1, G], f32, tag="t6", name="t6")

            def emit_body():
              for bt, halves in ((0, (0, 1)), (1, (2, 3))):
                  W = BTW[bt]
                  attT = attTb[bt]
                  av = attT.rearrange("q (c g) -> q c g", g=W)
                  for hh, h in enumerate(halves):
                      # ---- F = x @ v12 for this half (128 graphs) ----
                      fp = psF.tile([N, 256], f32, tag="fp")
                      for sub in range(4):
                          st = 4 * h + sub
                          xT = xpool.tile([N, STR], fp8, tag="xT")
                          nc.sync.dma_start(xT[:], X[st])
                          for g in range(STG):
                              gg = sub * STG + g
                              nc.tensor.matmul(
                                  fp[:, 2 * gg:2 * gg + 2],
                                  xT[:, N * g:N * g + N],
                                  vvb[:],
                                  start=True, stop=True,
                              )
                      fh = fhpool.tile([N, 256], fp16, tag="fh")
                      nc.scalar.copy(fh[:], fp[:])
                      fgp = psFg.tile([128, 2 * N], fp16, tag="fgp")
                      nc.tensor.transpose(fgp[:, 0:N], fh[:, 0::2], i116[:])
                      nc.tensor.transpose(fgp[:, N:2 * N], fh[:, 1::2], i116[:])
                      f1t = fpool.tile([128, N], fp16, tag="f1t", name="f1t")
                      f2t = fpool.tile([128, N], fp16, tag="f2t", name="f2t")
                      nc.vector.tensor_copy(f1t[:], fgp[:, 0:N])
                      nc.vector.tensor_copy(f2t[:], fgp[:, N:2 * N])

                      # ---- z, mask, exp-domain lrelu ----
                      p = ppool.tile([128, NN], fp16, tag="p")
                      for s in range(NSTRIP):
                          msb = mpool.tile([128, SW], fp16, tag="msb")
                          nc.sync.dma_start(msb[:], MADJ[h][:, s * SW:(s + 1) * SW])
                          z = zpool.tile([128, SW], fp16, tag="z")
                          f1v = (f1t[:, s * SI:(s + 1) * SI]
                                 .rearrange("q (i u) -> q i u", u=1)
                                 .broadcast_to([128, SI, N]))
                          f2v = (f2t[:]
                                 .rearrange("q (u j) -> q u j", u=1)
                                 .broadcast_to([128, SI, N]))
                          zv = z.rearrange("q (i j) -> q i j", j=N)
                          nc.gpsimd.tensor_tensor(zv, f1v, f2v, op=AL.add)
                          nc.vector.tensor_tensor(z[:], z[:], msb[:], op=AL.add)
                          ps = p[:, s * SW:(s + 1) * SW]
                          nc.scalar.activation(ps, z[:], AF.Exp)
                          nc.scalar.activation(z[:], z[:], AF.Exp, scale=0.2)
                          nc.vector.tensor_tensor(ps, ps, z[:], op=AL.max)

                      # ---- per-strip: fp16 tree-sum, 16/s recip, fp8 norm ----
                      pv = p.rearrange("q (i j) -> q i j", j=N)
                      pn = pnpool.tile([128, KPAD], fp8, tag="pn")
                      nc.gpsimd.memset(pn[:, NN:KPAD], 0.0)
                      sb = srpool.tile([128, N], f32, tag="sb", name="sb")
                      rb = srpool.tile([128, N], f32, tag="rb", name="rb")
                      for s in range(NSTRIP):
                          i0, i1 = s * SI, (s + 1) * SI
                          pv_s = pv[:, i0:i1, :]
                          t1 = t1pool.tile([128, SI * 58], fp16, tag="t1")
                          t1v = t1.rearrange("q (i j) -> q i j", j=58)
                          nc.vector.tensor_tensor(
                              t1v, pv_s[:, :, 0:58], pv_s[:, :, 58:116],
                              op=AL.add)
                          t2 = t2pool.tile([128, SI * 29], fp16, tag="t2")
                          t2v = t2.rearrange("q (i j) -> q i j", j=29)
                          nc.vector.tensor_tensor(
                              t2v, t1v[:, :, 0:29], t1v[:, :, 29:58], op=AL.add)
                          sbs = sb[:, i0:i1]
                          rbs = rb[:, i0:i1]
                          nc.vector.tensor_reduce(
                              sbs, t2v, axis=AX.X, op=AL.add)
                          nc.vector.tensor_scalar_mul(sbs, sbs, 1.0 / ATTSCALE)
                          nc.vector.reciprocal(rbs, sbs)
                          pnv_s = (pn[:, s * SW:(s + 1) * SW]
                                   .rearrange("q (i j) -> q i j", j=N))
                          rv = (rbs.rearrange("q (i u) -> q i u", u=1)
                                .broadcast_to([128, SI, N]))
                          eng = nc.gpsimd if (h, s) in NORM_POOL else nc.vector
                          eng.tensor_tensor(pnv_s, pv_s, rv, op=AL.mult)

                      # ---- transpose chunks + evac into attT ----
                      # fp8 PE transpose writes with element step 2 at 4B-
                      # aligned starts (hw rule): each chunk gets its own 256B
                      # PSUM region, values at even bytes; evac copies do the
                      # stride removal into dense fp8 attT.
                      gofs = 128 * hh
                      done = 0
                      bidx = 0
                      while done < NKC:
                          hi = min(done + 8, NKC)
                          tp = psT.tile([128, 2048], fp8, tag="tp")
                          tv = tp.rearrange("q (c b) -> q c b", b=256)
                          for c in range(done, hi):
                              nc.tensor.transpose(
                                  tv[:, c - done, 0::2],
                                  pn[:, 128 * c:128 * c + 128],
                                  i128[:])
                          dstv = av[:, done:hi, gofs:gofs + 128]
                          srcv = tv[:, 0:hi - done, 0::2]
                          if bidx % 7 in EVAC_DVE:
                              nc.vector.tensor_copy(dstv, srcv)
                          else:
                              nc.scalar.copy(dstv, srcv)
                          done = hi
                          bidx += 1

                  # ---- MLP pass for this batch-tile (W graphs) ----
                  h2a = psH2.tile([1, W], f32, tag="h2a", name="h2a")
                  h2b = psH2.tile([1, W], f32, tag="h2b", name="h2b")
                  for oc in range(OC):
                      hp = psH.tile([128, W], f32, tag="hp")
                      for (c0, npair) in SLABS:
                          w1s = wpool.tile([128, npair * 2 * 128], fp8, tag="w1s")
                          weng = nc.scalar if bt == 0 else nc.sync
                          weng.dma_start(
                              w1s[:],
                              W1[oc][:, c0 * 128:(c0 + 2 * npair) * 128])
                          w1v = w1s.rearrange("q (c m) -> q c m", m=128)
                          for pr in range(npair):
                              cc = c0 + 2 * pr
                              nc.tensor.matmul(
                                  hp[:],
                                  w1v[:, 2 * pr:2 * pr + 2, :],
                                  av[:, cc:cc + 2, :],
                                  start=(cc == 0), stop=(cc + 2 == NKC),
                                  perf_mode=PM.DoubleRow,
                              )
                      h1 = hpool.tile([128, W], bf16, tag="h1", name="h1")
                      nc.scalar.activation(
                          h1[:], hp[:], AF.Relu,
                          bias=b1s[:, oc:oc + 1], scale=H1SCALE)
                      nc.tensor.matmul(
                          h2a[:], w2s[:, 2 * oc:2 * oc + 1], h1[:],
                          start=(oc == 0), stop=(oc == OC - 1),
                      )
                      nc.tensor.matmul(
                          h2b[:], w2s[:, 2 * oc + 1:2 * oc + 2], h1[:],
                          start=(oc == 0), stop=(oc == OC - 1),
                      )
                  g0 = 256 * bt
                  nc.scalar.activation(ca[:, g0:g0 + W], h2a[:],
                                       AF.Identity, bias=b2s[0:1, 0:1], scale=1.0)
                  nc.scalar.activation(cb[:, g0:g0 + W], h2b[:],
                                       AF.Identity, bias=b2s[0:1, 1:2], scale=1.0)

              # ---- log_softmax over the 2 classes ----
              nc.vector.tensor_tensor(t6[:], ca[:], cb[:], op=AL.max)
              nc.vector.tensor_tensor(ca[:], ca[:], t6[:], op=AL.subtract)
              nc.vector.tensor_tensor(cb[:], cb[:], t6[:], op=AL.subtract)
              ea = psH.tile([1, G], f32, tag="hp", name="ea")
              nc.scalar.activation(ea[:], ca[:], AF.Exp)
              nc.scalar.activation(t6[:], cb[:], AF.Exp)
              nc.vector.tensor_tensor(t6[:], t6[:], ea[:], op=AL.add)
              nc.scalar.activation(t6[:], t6[:], AF.Ln)
              nc.vector.tensor_tensor(ca[:], ca[:], t6[:], op=AL.subtract)
              nc.vector.tensor_tensor(cb[:], cb[:], t6[:], op=AL.subtract)
              nc.sync.dma_start(OUT[0:1, :], ca[:])
              nc.sync.dma_start(OUT[1:2, :], cb[:])

            if reps == 1:
                emit_body()
            else:
                with tc.For_i(0, reps, 1) as _i:
                    emit_body()

    return nc


def _prep_inputs(x, adj, W_att, a1, a2, W1, b1, W2, b2):
    x = np.asarray(x, np.float32)
    adj = np.asarray(adj, np.float32)
    xf8 = x.astype(FP8)
    xr = np.ascontiguousarray(
        xf8.reshape(NCORES, NST, STG, N, N).transpose(0, 1, 4, 2, 3)
        .reshape(NCORES, NST, N, STR))
    madj = np.where(adj > 0, np.float16(0.0), np.float16(MASKVAL))
    madjr = np.ascontiguousarray(
        madj.reshape(NCORES, NHALF, GH, NN)).astype(np.float16)
    v12 = (np.asarray(W_att, np.float64)
           @ np.stack([np.asarray(a1, np.float64), np.asarray(a2, np.float64)], 1))
    vvb = np.ascontiguousarray(v12.astype(np.float32).astype(BF16))
    W1p = np.zeros((KPAD, 1024), np.float32)
    W1p[:NN] = np.asarray(W1, np.float32) * W1SCALE
    w1r = np.ascontiguousarray(
        W1p.reshape(NKC, 128, OC, 128).transpose(2, 1, 0, 3)
        .reshape(OC, 128, NKC * 128)).astype(FP8)
    w2r = np.ascontiguousarray(
        np.asarray(W2, np.float32).reshape(OC, 128, 2)
        .transpose(1, 0, 2).reshape(128, 16)).astype(BF16)
    b1r = np.ascontiguousarray(np.asarray(b1, np.float32).reshape(OC, 128).T)
    b2c = np.ascontiguousarray(np.asarray(b2, np.float32).reshape(1, 2))
    consts = dict(
        w1=w1r, w2=w2r, b1r=b1r, b2c=b2c, vvb=vvb,
        i128b=np.eye(128, dtype=FP8),
        i116h=np.eye(N, dtype=np.float16),
    )
    return [dict(consts, x=xr[c], madj=madjr[c]) for c in range(NCORES)]


TRACE = False
LAST_RESULTS = None


def kernel(x, adj, W_att, a1, a2, W1, b1, W2, b2):
    global LAST_RESULTS
    in_maps = _prep_inputs(x, adj, W_att, a1, a2, W1, b1, W2, b2)
    nc = build_nc()
    nc.compile()
    bres = run_bass_kernel_spmd(nc, in_maps, list(range(NCORES)), trace=TRACE)
    LAST_RESULTS = bres
    res = bres.results
    out = np.empty((B, 2), np.float32)
    for c in range(NCORES):
        out[c * G:(c + 1) * G] = np.asarray(res[c]["out"]).T
    return out
